# revision 19
# baseline (speedup 1.0000x reference)
"""Trainium2 Bass kernel for nn_ContrastiveLoss (B=512, D=256, 8 cores).

Math: with z = l2norm(rows), reps = concat(z_i, z_j) [512,256], the loss
splits into a positive term (same-label pairs, d^2) and a negative term
(relu(2.5-d)^2).  The positive term has an exact rank-40 closed form via
per-class sums, computed on host in float64:
  sum_p = 4*[2*(sum_c n_c^2 - B) - 2*(sum_c ||s_c||^2 - B)]
The negative term needs the full O(B^2) distance matrix -> device.

Device: G = Z Z^T (bf16, fp32 PSUM).  relu(2.5-d)^2 with d = 2*sqrt(2-2G)
is active iff G > 0.21875, and on the clamped value
  w = clamp(G, 0.21875, 1.0),  s = sqrt(8 - 8w)   (s = d when active)
the per-pair term is LINEAR in (w, s):
  relu(2.5-d)^2 = 6.25 - 5s + s^2 = 14.25 - 5s - 8w   (0 when inactive)
so only the row-sums of w and s are needed: one DVE clamp with accum and
one ACT sqrt with accum.  The device result includes the positive pairs
and the diagonal; the host subtracts exactly those terms, simulated from
the same bf16 operands (fp32), and adds the closed-form positive loss.

Sharding: 512 b-rows split 8 ways (64/core); per core two [128, 576]
bf16 DMAs (moving operand + stationary slab packed together), two
accumulating matmuls, two pointwise ops, one [64,2] f32 DMA out.
"""

import numpy as np
import ml_dtypes

import concourse.bass as bass
import concourse.mybir as mybir
import concourse.tile as tile
from concourse.bass_utils import run_bass_kernel_spmd

F32 = mybir.dt.float32
BF16 = mybir.dt.bfloat16
AF = mybir.ActivationFunctionType
OP = mybir.AluOpType

B = 512
D = 256
HALF = 256
NCORES = 8
BC = B // NCORES  # 64 b-rows per core
NCLS = 40
GLO = 0.21875     # relu active iff G > GLO;  8 - 8*GLO = 6.25
GHI = 1.0

TRACE = False
LAST_RESULT = None
_NC_CACHE = None


def _split_multi_waits(nc):
    """This walrus build allows only ONE sync-wait per instruction; Tile can
    attach several.  Hoist extras onto NoOps inserted before the owner."""
    cnt = 0
    for f in nc.m.functions:
        for bb in f.blocks:
            il = bb.instructions
            i = 0
            while i < len(il):
                ins = il[i]
                si = ins.sync_info
                if si is not None and len(si.on_wait) > 1:
                    waits = list(si.on_wait)
                    si.on_wait = [waits[-1]]
                    ins.sync_info = si
                    for w in waits[:-1]:
                        cnt += 1
                        nop = mybir.InstNoOp(
                            name=f"hoistw-{cnt}", ins=[], outs=[],
                            sync_info=type(si)(on_wait=[w], on_update=[]),
                        )
                        nop.engine = ins.engine
                        il.insert(i, nop)
                        i += 1
                i += 1
    return cnt


def _build():
    nc = bass.Bass(target_bir_lowering=False, debug=False)
    # Per-core packed inputs: moving operand [128, 512] next to its
    # stationary slab [128, 64] so each K-chunk arrives in ONE DMA.
    k0 = nc.dram_tensor("k0", [128, B + BC], BF16, kind="ExternalInput")
    k1 = nc.dram_tensor("k1", [128, B + BC], BF16, kind="ExternalInput")
    out = nc.dram_tensor("out", [128, 2], F32, kind="ExternalOutput")

    with tile.TileContext(nc) as tc:
        with (
            tc.tile_pool(name="sb", bufs=1) as sb,
            tc.tile_pool(name="ps", bufs=1, space="PSUM") as ps,
        ):
            # warm-up: pull in the sqrt_and_others ACT table set under the
            # DMA shadow
            warm = sb.tile([1, 1], F32, tag="warm")
            nc.gpsimd.memset(warm[:], 1.0)
            nc.scalar.activation(warm[:], warm[:], AF.Sqrt)
            c8 = sb.tile([128, 1], F32, tag="c8")
            nc.vector.memset(c8[:], 8.0)
            ones = sb.tile([128, B // 2], F32, tag="ones")
            nc.vector.memset(ones[:], 1.0)

            t0 = sb.tile([128, B + BC], BF16, tag="t0")
            nc.sync.dma_start(t0[:], k0[:, :])
            t1 = sb.tile([128, B + BC], BF16, tag="t1")
            nc.gpsimd.dma_start(t1[:], k1[:, :])

            # G slab as [128, 256]: partitions 0:64 = b-rows x a-cols 0:256,
            # partitions 64:128 = same b-rows x a-cols 256:512.  Full-lane
            # pointwise, 4 accumulating matmuls (2 K-chunks x 2 a-halves).
            HB = B // 2
            ps_m = ps.tile([128, HB], F32, tag="ps_m")
            for h in range(2):
                pslab = ps_m[64 * h:64 * (h + 1), :]
                mv = slice(HB * h, HB * (h + 1))
                nc.tensor.matmul(pslab, t0[:, B:B + BC], t0[:, mv],
                                 start=True, stop=False)
                nc.tensor.matmul(pslab, t1[:, B:B + BC], t1[:, mv],
                                 start=False, stop=True)

            # w = clamp(G, 0.21875, 1.0), rowsum -> part[:,0]
            # s = sqrt(8 - 8w),           rowsum -> part[:,1]
            # (the DVE tensor_scalar second ALU op and its accumulator are
            # broken in this build: one STT computes (GLO max G) min ones,
            # with the ones tensor supplying the 1.0 upper clamp, and its
            # accum_out is the proven accumulator path)
            part = sb.tile([128, 2], F32, tag="part")
            w = sb.tile([128, HB], F32, tag="w")
            nc.vector.scalar_tensor_tensor(w[:], ps_m[:], GLO, ones[:],
                                           OP.max, OP.min,
                                           accum_out=part[:, 0:1])
            s = sb.tile([128, HB], F32, tag="s")
            nc.scalar.activation(s[:], w[:], AF.Sqrt,
                                 bias=c8[:, 0:1], scale=-8.0,
                                 accum_out=part[:, 1:2])
            nc.scalar.dma_start(out[:, :], part[:])

    _split_multi_waits(nc)
    return nc


def kernel(**inputs):
    global _NC_CACHE, LAST_RESULT
    emb_i = np.asarray(inputs["emb_i"], dtype=np.float64)
    emb_j = np.asarray(inputs["emb_j"], dtype=np.float64)
    y = np.asarray(inputs["y"]).astype(np.int64)
    assert emb_i.shape == (HALF, D) and emb_j.shape == (HALF, D)

    # ---- host: normalize (f64) + exact closed-form positive term ----
    X = np.concatenate([emb_i, emb_j], axis=0)          # [512, 256]
    n = np.sqrt((X * X).sum(axis=1, keepdims=True))
    Z = X / np.maximum(n, 1e-12)
    ncls = np.bincount(y, minlength=NCLS).astype(np.float64)
    Hmat = np.zeros((max(NCLS, y.max() + 1), B))
    Hmat[y, np.arange(B)] = 1.0
    s_c = Hmat @ Z
    n_pos_incl_diag = float((ncls ** 2).sum())
    n_pos_offdiag = n_pos_incl_diag - B
    sum_same_d2 = 2.0 * n_pos_offdiag - 2.0 * ((s_c * s_c).sum() - B)
    # eps^2 term from F.pairwise_distance (linear term cancels by symmetry)
    sum_p = 4.0 * (sum_same_d2 + n_pos_offdiag * D * 1e-12)

    # ---- device operands (bf16) ----
    ZTb = np.ascontiguousarray(Z.T).astype(ml_dtypes.bfloat16)  # [256, 512]
    Zf = ZTb.astype(np.float32)

    # host simulation of the device's bf16 negative-term formula for the
    # pairs the device should NOT contribute: positives and the diagonal
    def _negterm(g):
        wv = np.clip(g, GLO, GHI)
        sv = np.sqrt(np.maximum(8.0 - 8.0 * wv, 0.0))
        return 14.25 - 5.0 * sv - 8.0 * wv

    sim_posdiag = 0.0
    for c in range(int(y.max()) + 1):
        idx = np.nonzero(y == c)[0]
        if len(idx) == 0:
            continue
        Gc = Zf[:, idx].T @ Zf[:, idx]                 # fp32, incl diag
        sim_posdiag += float(_negterm(Gc).sum())

    if _NC_CACHE is None:
        _NC_CACHE = _build()
    nc = _NC_CACHE

    in_maps = []
    for c in range(NCORES):
        r0 = c * BC
        in_maps.append({
            "k0": np.ascontiguousarray(
                np.concatenate([ZTb[0:128, :], ZTb[0:128, r0:r0 + BC]], 1)),
            "k1": np.ascontiguousarray(
                np.concatenate([ZTb[128:256, :], ZTb[128:256, r0:r0 + BC]], 1)),
        })

    res = run_bass_kernel_spmd(nc, in_maps, core_ids=list(range(NCORES)),
                               trace=TRACE)
    LAST_RESULT = res
    sw = 0.0
    ss = 0.0
    for c in range(NCORES):
        o = res.results[c]["out"].astype(np.float64)
        sw += o[:, 0].sum()
        ss += o[:, 1].sum()
    dev_n = 14.25 * BC * B * NCORES - 5.0 * ss - 8.0 * sw
    loss = (sum_p + dev_n - sim_posdiag) / (2.0 * B)
    return np.float32(loss)
